# revision 41
# baseline (speedup 1.0000x reference)
"""Trainium2 Bass kernel for nn_Block_2018634629560 (dense transformer block:
gemma-normed gated attention + gated delta-net), 8-core tensor-parallel.

Two SPMD launches, head-sharded tensor parallel:
  Launch 1 (attention): 2 q-heads/core, kv-head replicated per pair; each
    core emits its partial o-projection [T, D] bf16; host reduces.
  Launch 2 (delta-net): 4 v-heads (2 k-heads)/core, chunked delta rule
    (chunk=128) with Horner solve of (I+A)^-1 ~ I-A+A^2-A^3.

v3 notes (on top of v2):
  - all six big GEMMs (qg/kv/o-proj, dn qkv/z/out-proj) run in fp8e4
    with DoubleRow perf mode (2 k-subtiles per matmul, 0.5 cyc/row).
    Weights are host-scaled by SW=1024 into fp8 range; compensation is
    folded into existing activation scales / host-side reduction.
  - x and h are RMS-normalized on the host before transpose+fp8; the
    per-token scale columns (s1/s2) and their device-side multiplies
    are gone (q/k/z norms are scale-invariant, the rest is compensated
    in folded constants).
  - activation-table discipline: one function family per program-order
    region (Sqrt+Square+Copy regions, Exp regions, one Silu region);
    Ln/Exp pairs are avoided because the table-load pass puts them in
    different tables and ping-pongs 1283ns loads.
  - PSUM->SBUF copies of projection outputs are spread across DVE/Act;
    SBUF-only elementwise work (gram masking, zs/og gating muls) is
    offloaded to GpSimd (which cannot touch PSUM or run TensorScalar).
"""
import math
import numpy as np
import ml_dtypes

import concourse.bass as bass
import concourse.tile as tile
from concourse import bacc, mybir
from concourse.bass import ts, ds
from concourse.bass_utils import run_bass_kernel_spmd

F32 = mybir.dt.float32
BF16 = mybir.dt.bfloat16
F8 = mybir.dt.float8e4
AF = mybir.ActivationFunctionType
ALU = mybir.AluOpType
DR = mybir.MatmulPerfMode.DoubleRow
BFNP = ml_dtypes.bfloat16
F8NP = ml_dtypes.float8_e4m3

# ---- problem constants ----
D = 2048; HQ = 16; HKV = 4; HD = 128; ROT = 32; THETA = 10000.0; EPS = 1e-6
HK = 16; HV = 32; DK = 128; DV = 128; KCONV = 4
KEY_DIM = HK * DK; VAL_DIM = HV * DV; CONV_DIM = 2 * KEY_DIM + VAL_DIM
B = 1; T = 2048
NCORE = 8
P = 128
TT = T // P      # 16 token tiles
KT = D // P      # 16 contraction tiles
KP = KT // 2     # 8 DoubleRow k-pairs
CH = 128         # delta chunk size
NCH = T // CH    # 16 chunks
SW = 1024.0      # fp8 weight pre-scale (power of 2)
ISW = 1.0 / SW


# ============================================================ launch 1 build
def build_attn():
    nc = bacc.Bacc("TRN2", target_bir_lowering=False, debug=False,
                   enable_asserts=False, num_devices=NCORE)
    dt = nc.dram_tensor
    xT = dt("xT", [P, KT, T], F8, kind="ExternalInput").ap()
    wqg = dt("wqg", [P, KT, 512], F8, kind="ExternalInput").ap()
    wkv = dt("wkv", [P, KT, 256], F8, kind="ExternalInput").ap()
    wo = dt("wo", [P, 2, D], F8, kind="ExternalInput").ap()
    csd = dt("csd", [P, TT, 96], F32, kind="ExternalInput").ap()
    qk1 = dt("qk1", [P, 256], BF16, kind="ExternalInput").ap()
    m4 = dt("m4", [P, 4 * 512], BF16, kind="ExternalInput").ap()
    idm = dt("idm", [P, P], BF16, kind="ExternalInput").ap()
    p1 = dt("p1", [T, D], BF16, kind="ExternalOutput").ap()

    with tile.TileContext(nc) as tc:
        with tc.tile_pool(name="res", bufs=1) as res:
            xT_sb = res.tile([P, KT, T], F8)
            wqg_sb = res.tile([P, KT, 512], F8)
            wkv_sb = res.tile([P, KT, 256], F8)
            cs_sb = res.tile([P, TT, 6, 16], F32)
            qk1_sb = res.tile([P, 256], BF16)
            m4_sb = res.tile([P, 4 * 512], BF16)
            id_sb = res.tile([P, P], BF16)
            wo_sb = res.tile([P, 2, D], F8)
            qT_sb = res.tile([P, 2, T], BF16)
            kT_sb = res.tile([P, T], BF16)
            vE_sb = res.tile([P, TT, 132], BF16)
            gs_sb = res.tile([P, TT, 256], F32)
            graw_sb = res.tile([P, TT, 256], BF16)
            ygT_sb = res.tile([P, 2, T], F8)
            eps_sb = res.tile([P, 1], F32)
            nc.vector.memset(eps_sb[:], EPS)

            # DMA order: first-needed first
            nc.sync.dma_start(wqg_sb[:], wqg[:])
            nc.sync.dma_start(wkv_sb[:], wkv[:])
            nc.sync.dma_start(qk1_sb[:], qk1[:])
            nc.sync.dma_start(cs_sb[:], csd[:])
            for q in range(4):
                nc.sync.dma_start(xT_sb[:, :, ts(q, 512)], xT[:, :, ts(q, 512)])
            nc.sync.dma_start(m4_sb[:], m4[:])
            nc.sync.dma_start(id_sb[:], idm[:])
            nc.sync.dma_start(wo_sb[:], wo[:])
            nc.vector.memset(vE_sb[:, :, 128:132], 0.0)
            nc.vector.memset(vE_sb[:, :, 128:129], 1.0)

            # ---------------- phase 1: projections + norms + rope ----------
            with tc.tile_pool(name="ph1", bufs=3) as ph1, \
                 tc.tile_pool(name="ph1s", bufs=8) as ph1s, \
                 tc.tile_pool(name="psqg", bufs=2, space="PSUM") as psqg, \
                 tc.tile_pool(name="pskv", bufs=2, space="PSUM") as pskv, \
                 tc.tile_pool(name="ptr", bufs=2, space="PSUM") as ptr:
                for i in range(TT):
                    pqg = psqg.tile([P, 512], F32)
                    pkv = pskv.tile([P, 256], F32)
                    for k in range(KP):
                        lhsT = xT_sb[:, 2 * k:2 * k + 2, ts(i, P)]
                        nc.tensor.matmul(pqg[:], lhsT,
                                         wqg_sb[:, 2 * k:2 * k + 2, :],
                                         start=(k == 0), stop=(k == KP - 1),
                                         perf_mode=DR)
                        nc.tensor.matmul(pkv[:], lhsT,
                                         wkv_sb[:, 2 * k:2 * k + 2, :],
                                         start=(k == 0), stop=(k == KP - 1),
                                         perf_mode=DR)
                    qn3 = ph1.tile([P, 3, 128], F32, tag="qn3")
                    for hh, (src, qkcol) in enumerate(
                            [(pqg[:, 0:128], 0), (pqg[:, 128:256], 0),
                             (pkv[:, 0:128], 128)]):
                        sq2 = ph1.tile([P, 128], F32, tag="sq2")
                        ss2 = ph1s.tile([P, 1], F32, tag="ss2")
                        nc.scalar.activation(sq2[:], src, AF.Square,
                                             accum_out=ss2[:])
                        sl = ph1s.tile([P, 1], F32, tag="sl")
                        nc.scalar.activation(sl[:], ss2[:], AF.Sqrt,
                                             scale=1.0 / HD, bias=eps_sb[:])
                        rn = ph1s.tile([P, 1], F32, tag="rn")
                        nc.vector.reciprocal(rn[:], sl[:])
                        nc.vector.scalar_tensor_tensor(
                            qn3[:, hh, :], src, rn[:],
                            qk1_sb[:, qkcol:qkcol + 128], ALU.mult, ALU.mult)
                    # rope on first 32 dims of all 3 heads at once
                    cos3 = cs_sb[:, i, 0:3, :]
                    sin3 = cs_sb[:, i, 3:6, :]
                    x1 = ph1s.tile([P, 3, 16], F32, tag="x1")
                    x2 = ph1s.tile([P, 3, 16], F32, tag="x2")
                    nc.vector.tensor_copy(x1[:], qn3[:, :, 0:16])
                    nc.vector.tensor_copy(x2[:], qn3[:, :, 16:32])
                    t1 = ph1s.tile([P, 3, 16], F32, tag="t1")
                    t2 = ph1s.tile([P, 3, 16], F32, tag="t2")
                    nc.vector.tensor_mul(t1[:], x1[:], cos3)
                    nc.vector.tensor_mul(t2[:], x2[:], sin3)
                    nc.vector.tensor_sub(qn3[:, :, 0:16], t1[:], t2[:])
                    nc.vector.tensor_mul(t1[:], x2[:], cos3)
                    nc.vector.tensor_mul(t2[:], x1[:], sin3)
                    nc.vector.tensor_add(qn3[:, :, 16:32], t1[:], t2[:])
                    qnb = ph1.tile([P, 3, 128], BF16, tag="qnb")
                    nc.vector.tensor_copy(qnb[:], qn3[:])
                    ptt = ptr.tile([P, 3, 128], BF16)
                    for hh in range(3):
                        nc.tensor.transpose(ptt[:, hh, :], qnb[:, hh, :],
                                            id_sb[:])
                    nc.vector.tensor_copy(qT_sb[:, :, ts(i, P)], ptt[:, 0:2, :])
                    nc.vector.tensor_copy(kT_sb[:, ts(i, P)], ptt[:, 2, :])
                    # v (compensate fp8 weight scale); gate staged for the
                    # exp-region sigmoid pass
                    nc.vector.tensor_scalar(
                        vE_sb[:, i, 0:128], pkv[:, 128:256], ISW,
                        None, ALU.mult)
                    nc.vector.tensor_copy(graw_sb[:, i, :], pqg[:, 256:512])

            # gate sigmoid via exp+recip (start of the exp-table region)
            with tc.tile_pool(name="sg", bufs=3) as sgp:
                for i in range(TT):
                    ge = sgp.tile([P, 256], F32, tag="ge")
                    nc.scalar.activation(ge[:], graw_sb[:, i, :], AF.Exp,
                                         scale=-ISW)
                    ge1 = sgp.tile([P, 256], F32, tag="ge1")
                    nc.vector.tensor_scalar_add(ge1[:], ge[:], 1.0)
                    nc.vector.reciprocal(gs_sb[:, i, :], ge1[:])

            # ---------------- phase 2: attention core ----------------------
            with tc.tile_pool(name="expp", bufs=20) as expp, \
                 tc.tile_pool(name="ph2", bufs=4) as ph2, \
                 tc.tile_pool(name="ph2s", bufs=4) as ph2s, \
                 tc.tile_pool(name="psT", bufs=2, space="PSUM") as psT, \
                 tc.tile_pool(name="psy", bufs=2, space="PSUM") as psy, \
                 tc.tile_pool(name="ptr2", bufs=2, space="PSUM") as ptr2:
                for h in range(2):
                    for J in range(4):
                        expTs = []
                        for i2 in range(4 * J + 4):
                            pT = psT.tile([P, 512], F32)
                            nc.tensor.matmul(
                                pT[:], kT_sb[:, ts(i2, P)],
                                qT_sb[:, h, ts(J, 512)],
                                start=True, stop=True)
                            et = expp.tile([P, 512], BF16, tag="expT")
                            nc.scalar.activation(et[:], pT[:], AF.Exp,
                                                 scale=1.0 / math.sqrt(HD))
                            r = i2 - 4 * J
                            if r >= 0:
                                nc.vector.tensor_mul(
                                    et[:], et[:], m4_sb[:, ts(r, 512)])
                            expTs.append(et)
                        for m in range(4 * J, 4 * J + 4):
                            py = psy.tile([P, 132], F32)
                            for i2 in range(m + 1):
                                nc.tensor.matmul(
                                    py[:, 0:129],
                                    expTs[i2][:, ts(m - 4 * J, P)],
                                    vE_sb[:, i2, 0:129],
                                    start=(i2 == 0), stop=(i2 == m))
                            rd = ph2s.tile([P, 1], F32, tag="rd")
                            nc.vector.reciprocal(rd[:], py[:, 128:129])
                            yg = ph2.tile([P, P], BF16, tag="yg")
                            nc.vector.scalar_tensor_tensor(
                                yg[:], py[:, 0:128], rd[:],
                                gs_sb[:, m, ts(h, P)], ALU.mult, ALU.mult)
                            pt2 = ptr2.tile([P, P], BF16)
                            nc.tensor.transpose(pt2[:], yg[:], id_sb[:])
                            nc.vector.tensor_copy(ygT_sb[:, h, ts(m, P)],
                                                  pt2[:])

            # ---------------- phase 3: o-projection (fp8 DoubleRow) --------
            with tc.tile_pool(name="ph3", bufs=3) as ph3, \
                 tc.tile_pool(name="pso", bufs=4, space="PSUM") as pso:
                for m in range(TT):
                    ob = ph3.tile([P, D], BF16, tag="ob")
                    for n in range(4):
                        po = pso.tile([P, 512], F32)
                        nc.tensor.matmul(po[:], ygT_sb[:, 0:2, ts(m, P)],
                                         wo_sb[:, 0:2, ts(n, 512)],
                                         start=True, stop=True, perf_mode=DR)
                        if n % 2 == 1:
                            nc.scalar.activation(ob[:, ts(n, 512)], po[:],
                                                 AF.Copy)
                        else:
                            nc.vector.tensor_copy(ob[:, ts(n, 512)], po[:])
                    nc.sync.dma_start(p1[ts(m, P), :], ob[:])
    nc.compile()
    return nc


# ============================================================ launch 2 build
def build_delta():
    nc = bacc.Bacc("TRN2", target_bir_lowering=False, debug=False,
                   enable_asserts=False, num_devices=NCORE)
    dt = nc.dram_tensor
    hT = dt("hT", [P, KT, T], F8, kind="ExternalInput").ap()
    wqkv = dt("wqkv", [P, KT, 1024], F8, kind="ExternalInput").ap()
    cwt = dt("cwt", [P, 8 * KCONV], F32, kind="ExternalInput").ap()
    wz = dt("wz", [P, KT, 512], F8, kind="ExternalInput").ap()
    wab = dt("wab", [P, KT, 8], F8, kind="ExternalInput").ap()
    wout = dt("wout", [P, 4, D], F8, kind="ExternalInput").ap()
    dtb = dt("dtb", [P, 4], F32, kind="ExternalInput").ap()
    nega = dt("nega", [P, TT, 4], F32, kind="ExternalInput").ap()
    nwbc = dt("nwbc", [P, 512], BF16, kind="ExternalInput").ap()
    msku = dt("msku", [P, 4, P], BF16, kind="ExternalInput").ap()
    mskud = dt("mskud", [P, 4, P], BF16, kind="ExternalInput").ap()
    blkd = dt("blkd", [4, 512], F32, kind="ExternalInput").ap()
    idb = dt("idb", [P, P], BF16, kind="ExternalInput").ap()
    idf = dt("idf", [P, P], F32, kind="ExternalInput").ap()
    cums = dt("cums", [4, T], F32, kind="Internal").ap()
    p2 = dt("p2", [T, D], BF16, kind="ExternalOutput").ap()

    with tile.TileContext(nc) as tc:
      with tc.tile_pool(name="res", bufs=1) as res:
        qkv_sb = res.tile([P, 8, T], BF16)      # conv+silu outputs [f, t]
        zs_sb = res.tile([P, TT, 512], BF16)    # silu(z)*nw [t, f]
        S_sb = res.tile([P, 4, DV], F32)
        S_bf = res.tile([P, 4, DV], BF16)
        beta_sb = res.tile([P, TT, 4], F32)
        nbeta_sb = res.tile([P, TT, 4], F32)
        g_sb = res.tile([P, TT, 4], F32)
        ta2_sb = res.tile([P, TT, 4], F32)
        bbs_sb = res.tile([P, TT, 4], F32)
        cw_sb = res.tile([P, 8, KCONV], F32)
        dtb_sb = res.tile([P, 4], F32)
        nega_sb = res.tile([P, TT, 4], F32)
        nw_sb = res.tile([P, 512], BF16)
        idb_sb = res.tile([P, P], BF16)
        idf_sb = res.tile([P, P], F32)
        ones1b = res.tile([1, P], BF16)
        onesA = res.tile([P, P], F32)           # all-ones; row slices as lhsT
        onescolb = res.tile([P, 1], BF16)
        one_c = res.tile([P, 1], F32)
        tiny_c = res.tile([P, 1], F32)
        eps_c = res.tile([P, 1], F32)

        nc.vector.memset(S_sb[:], 0.0)
        nc.vector.memset(S_bf[:], 0.0)
        nc.vector.memset(ones1b[:], 1.0)
        nc.vector.memset(onesA[:], 1.0)
        nc.vector.memset(onescolb[:], 1.0)
        nc.vector.memset(one_c[:], 1.0)
        nc.vector.memset(tiny_c[:], 1e-20)
        nc.vector.memset(eps_c[:], EPS)

        nc.sync.dma_start(cw_sb[:], cwt[:])
        nc.sync.dma_start(dtb_sb[:], dtb[:])
        nc.sync.dma_start(idb_sb[:], idb[:])
        nc.sync.dma_start(idf_sb[:], idf[:])

        # ======== B/C/D: projections + conv + z/ab (Silu region) ==========
        with tc.tile_pool(name="big1", bufs=1) as big1, \
             tc.tile_pool(name="hTp", bufs=3) as hTp, \
             tc.tile_pool(name="mxp", bufs=10) as mxp, \
             tc.tile_pool(name="wk1", bufs=4) as wk1, \
             tc.tile_pool(name="psB", bufs=2, space="PSUM") as psB, \
             tc.tile_pool(name="psab", bufs=2, space="PSUM") as psab:
            wqkv_sb = big1.tile([P, KT, 1024], F8)
            wz_sb = big1.tile([P, KT, 512], F8)
            wab_sb = big1.tile([P, KT, 8], F8)
            nc.sync.dma_start(wqkv_sb[:, :, 0:512], wqkv[:, :, 0:512])
            hT0 = hTp.tile([P, KT, 512], F8, tag="hTn")
            nc.sync.dma_start(hT0[:], hT[:, :, ts(0, 512)])
            nc.sync.dma_start(wqkv_sb[:, :, 512:1024], wqkv[:, :, 512:1024])
            hT1 = hTp.tile([P, KT, 512], F8, tag="hTn")
            nc.sync.dma_start(hT1[:], hT[:, :, ts(1, 512)])
            nc.sync.dma_start(wz_sb[:], wz[:])
            nc.sync.dma_start(wab_sb[:], wab[:])
            nc.sync.dma_start(nega_sb[:], nega[:])
            nc.sync.dma_start(nw_sb[:], nwbc[:])
            hT_tiles = [hT0, hT1]

            prev_mx = [None] * 8
            for n4 in range(4):
                if n4 + 2 < 4:
                    hTn2 = hTp.tile([P, KT, 512], F8, tag="hTn")
                    nc.sync.dma_start(hTn2[:], hT[:, :, ts(n4 + 2, 512)])
                    hT_tiles.append(hTn2)
                hT_n = hT_tiles[n4]
                for F in range(8):
                    pm = psB.tile([P, 512], F32, tag="pm")
                    for k in range(KP):
                        nc.tensor.matmul(pm[:],
                                         wqkv_sb[:, 2 * k:2 * k + 2, ts(F, P)],
                                         hT_n[:, 2 * k:2 * k + 2, :],
                                         start=(k == 0), stop=(k == KP - 1),
                                         perf_mode=DR)
                    m1 = mxp.tile([P, 515], BF16, tag="mxc")
                    nc.scalar.activation(m1[:, 3:515], pm[:], AF.Copy)
                    if n4 == 0:
                        nc.vector.memset(m1[:, 0:3], 0.0)
                    else:
                        nc.vector.tensor_copy(m1[:, 0:3],
                                              prev_mx[F][:, 512:515])
                    prev_mx[F] = m1
                    # conv taps: tap0 on Act (Copy w/ scale), taps 1-3 as
                    # chained STT on DVE (STT is DVE-only)
                    c0 = wk1.tile([P, 512], BF16, tag="cc0")
                    nc.scalar.activation(c0[:], m1[:, 0:512], AF.Copy,
                                         scale=cw_sb[:, F, 0:1])
                    for j in range(1, KCONV):
                        c1 = wk1.tile([P, 512], BF16, tag=f"cc{j % 2}")
                        nc.vector.scalar_tensor_tensor(
                            c1[:], m1[:, j:512 + j], cw_sb[:, F, j:j + 1],
                            c0[:], ALU.mult, ALU.add)
                        c0 = c1
                    nc.scalar.activation(qkv_sb[:, F, ts(n4, 512)], c0[:],
                                         AF.Silu)
                for m in range(4 * n4, 4 * n4 + 4):
                    pz = psB.tile([P, 512], F32, tag="pm")
                    pab = psab.tile([P, 8], F32)
                    for k in range(KP):
                        lhsT = hT_n[:, 2 * k:2 * k + 2, ts(m - 4 * n4, P)]
                        nc.tensor.matmul(pz[:], lhsT,
                                         wz_sb[:, 2 * k:2 * k + 2, :],
                                         start=(k == 0), stop=(k == KP - 1),
                                         perf_mode=DR)
                        nc.tensor.matmul(pab[:], lhsT,
                                         wab_sb[:, 2 * k:2 * k + 2, :],
                                         start=(k == 0), stop=(k == KP - 1),
                                         perf_mode=DR)
                    zsg = wk1.tile([P, 512], BF16, tag="zsg")
                    nc.scalar.activation(zsg[:], pz[:], AF.Silu, scale=ISW)
                    nc.gpsimd.tensor_mul(zs_sb[:, m, :], zsg[:], nw_sb[:])
                    nc.vector.scalar_tensor_tensor(
                        ta2_sb[:, m, :], pab[:, 0:4], ISW,
                        dtb_sb[:], ALU.mult, ALU.add)
                    nc.vector.tensor_scalar(bbs_sb[:, m, :], pab[:, 4:8],
                                            ISW, None, ALU.mult)

        # ======== E/F working set (exp/ln region) ==========================
        with tc.tile_pool(name="ef", bufs=1) as ef:
            ogT_sb = ef.tile([P, 4, T], F8)
            wout_sb = ef.tile([P, 4, D], F8)
            cumT4 = ef.tile([4, T], F32)    # global cumsum rows (p0-3)
            ncumT4 = ef.tile([4, T], F32)   # negated
            blkd_sb = ef.tile([4, 4, P], F32)
            rkT_sb = ef.tile([1, 2, T], F32)
            rq_sb = ef.tile([P, NCH, 2], F32)
            ssqo_sb = ef.tile([P, NCH, 4], F32)
            o_sb = ef.tile([P, NCH, 4, DV], BF16)
            msku_sb = ef.tile([P, 4, P], BF16)
            mskud_sb = ef.tile([P, 4, P], BF16)
            nc.sync.dma_start(msku_sb[:], msku[:])
            nc.sync.dma_start(mskud_sb[:], mskud[:])
            nc.sync.dma_start(blkd_sb[:], blkd[:])
            nc.sync.dma_start(wout_sb[:], wout[:])

            # ---- scalar prep: beta, g, global cumsum ----------------------
            with tc.tile_pool(name="sp1", bufs=2) as sp1, \
                 tc.tile_pool(name="ptg", bufs=2, space="PSUM") as ptg:
                e3 = sp1.tile([P, TT, 4], F32, tag="e3")
                nc.scalar.activation(e3[:], ta2_sb[:], AF.Exp)
                e2 = sp1.tile([P, TT, 4], F32, tag="e2")
                nc.scalar.activation(e2[:], bbs_sb[:], AF.Exp, scale=-1.0)
                spl = sp1.tile([P, TT, 4], F32, tag="spl")
                nc.scalar.activation(spl[:], e3[:], AF.Ln, bias=one_c[:])
                nc.vector.tensor_mul(g_sb[:], spl[:], nega_sb[:])
                d2 = sp1.tile([P, TT, 4], F32, tag="d2")
                nc.vector.tensor_scalar_add(d2[:], e2[:], 1.0)
                nc.vector.reciprocal(beta_sb[:], d2[:])
                nc.vector.tensor_scalar_mul(nbeta_sb[:], beta_sb[:], -1.0)
                gT = sp1.tile([4, T], F32, tag="gT")
                for m in range(TT):
                    pt = ptg.tile([4, P], F32)
                    nc.tensor.transpose(pt[:], g_sb[:, m, :], idf_sb[:])
                    nc.vector.tensor_copy(gT[:, ts(m, P)], pt[:])
                nc.vector.tensor_tensor_scan(cumT4[:], gT[:], gT[:],
                                             0.0, ALU.add, ALU.bypass)
                nc.vector.tensor_scalar_mul(ncumT4[:], cumT4[:], -1.0)
                nc.sync.dma_start(cums[:], cumT4[:])

            # ---- D3: rk rows / rq cols (rsqrt via exp/ln) -----------------
            with tc.tile_pool(name="d3", bufs=4) as d3, \
                 tc.tile_pool(name="d3p", bufs=2, space="PSUM") as d3p:
                for n in range(NCH):
                    psr = d3p.tile([1, 2, P], F32, tag="psr")
                    psq = d3p.tile([P, 2], F32, tag="psq")
                    for kh in range(2):
                        sqk = d3.tile([P, P], BF16, tag="sqk")
                        nc.vector.tensor_mul(sqk[:],
                                             qkv_sb[:, 2 + kh, ts(n, P)],
                                             qkv_sb[:, 2 + kh, ts(n, P)])
                        nc.tensor.matmul(psr[:, kh, :], onescolb[:], sqk[:],
                                         start=True, stop=True)
                        sqq = d3.tile([P, P], BF16, tag="sqq")
                        nc.vector.tensor_mul(sqq[:], qkv_sb[:, kh, ts(n, P)],
                                             qkv_sb[:, kh, ts(n, P)])
                        nc.tensor.matmul(psq[:, kh:kh + 1], sqq[:],
                                         onescolb[:], start=True, stop=True)
                    lr = d3.tile([1, 2, P], F32, tag="lr")
                    nc.scalar.activation(lr[:], psr[:], AF.Sqrt,
                                         bias=tiny_c[0:1, :])
                    nc.vector.reciprocal(rkT_sb[:, :, ts(n, P)], lr[:])
                    lq = d3.tile([P, 2], F32, tag="lq")
                    nc.scalar.activation(lq[:], psq[:], AF.Sqrt,
                                         scale=float(DK), bias=tiny_c[:])
                    nc.vector.reciprocal(rq_sb[:, n, :], lq[:])

            # ---- phase E: chunked delta rule + gated out-proj -------------
            with tc.tile_pool(name="wkE", bufs=3) as wkE, \
                 tc.tile_pool(name="wkEs", bufs=4) as wkEs, \
                 tc.tile_pool(name="uP", bufs=6) as uP, \
                 tc.tile_pool(name="pE", bufs=1, space="PSUM") as pE, \
                 tc.tile_pool(name="pG", bufs=1, space="PSUM") as pG, \
                 tc.tile_pool(name="pX", bufs=1, space="PSUM") as pX, \
                 tc.tile_pool(name="pT", bufs=1, space="PSUM") as pT, \
                 tc.tile_pool(name="pC", bufs=2, space="PSUM") as pC, \
                 tc.tile_pool(name="pF", bufs=1, space="PSUM") as pF:
                prev_lrow = None
                for n in range(NCH):
                    # per-chunk decay scalars
                    sm = pX.tile([P, 20], F32, tag="sm")
                    nc.tensor.transpose(sm[:, 0:4], cumT4[:, ts(n, P)],
                                        idf_sb[0:4, 0:4])
                    cumc = wkEs.tile([P, 4], F32, tag="cumc")
                    nc.vector.tensor_copy(cumc[:], sm[:, 0:4])
                    nc.tensor.transpose(sm[0:1, 16:20],
                                        cumT4[:, n * P + 127:n * P + 128],
                                        idf_sb[0:4, 0:4])
                    cumR = wkEs.tile([1, 4, P], F32, tag="cumR")
                    nc.sync.dma_start(cumR[:], cums[:, ts(n, P)])
                    lrow = wkEs.tile([1, 4], F32, tag="lrow")
                    nc.vector.tensor_copy(lrow[:], sm[0:1, 16:20])
                    nc.tensor.matmul(sm[:, 8:12], onesA[0:1, :], lrow[:],
                                     start=True, stop=True)
                    lastb = wkEs.tile([P, 4], F32, tag="lastb")
                    nc.vector.tensor_copy(lastb[:], sm[:, 8:12])
                    di4 = wkEs.tile([P, 4], F32, tag="di4")
                    gend4 = wkEs.tile([P, 4], F32, tag="gend4")
                    if n == 0:
                        nc.scalar.activation(di4[:], cumc[:], AF.Exp)
                        nc.scalar.activation(gend4[:], lastb[:], AF.Exp)
                    else:
                        nc.tensor.matmul(sm[:, 4:8], onesA[0:1, :],
                                         prev_lrow[:], start=True, stop=True)
                        dloc = wkEs.tile([P, 4], F32, tag="dloc")
                        nc.vector.tensor_sub(dloc[:], cumc[:], sm[:, 4:8])
                        nc.scalar.activation(di4[:], dloc[:], AF.Exp)
                        ge = wkEs.tile([P, 4], F32, tag="ge")
                        nc.vector.tensor_sub(ge[:], lastb[:], sm[:, 4:8])
                        nc.scalar.activation(gend4[:], ge[:], AF.Exp)
                    prev_lrow = lrow
                    gr = wkEs.tile([P, 4], F32, tag="gr")
                    nc.vector.tensor_sub(gr[:], lastb[:], cumc[:])
                    grd4 = wkEs.tile([P, 4], F32, tag="grd4")
                    nc.scalar.activation(grd4[:], gr[:], AF.Exp)
                    diq4 = wkEs.tile([P, 4], F32, tag="diq4")
                    for kh in range(2):
                        nc.vector.tensor_scalar(
                            diq4[:, 2 * kh:2 * kh + 2],
                            di4[:, 2 * kh:2 * kh + 2],
                            rq_sb[:, n, kh:kh + 1], None, ALU.mult)
                    dnb4 = wkEs.tile([P, 4], F32, tag="dnb4")
                    nc.vector.tensor_mul(dnb4[:], di4[:], nbeta_sb[:, n, :])
                    # pairwise decay tiles exp(c_t - c_s) in [s, t] layout;
                    # causal masks folded in as -60000 bias tiles added via
                    # identity matmul (exp -> exact 0 in masked region), so
                    # no min-clamp / mask multiplies are needed.
                    pe4 = pE.tile([P, 4, P], F32, tag="pe4")
                    for hh in range(4):
                        nc.tensor.matmul(pe4[:, hh, :], ncumT4[:, ts(n, P)],
                                         blkd_sb[:, hh, :],
                                         start=True, stop=False)
                        nc.tensor.matmul(pe4[:, hh, :], onesA[0:1, :],
                                         cumR[0:1, hh, :],
                                         start=False, stop=False)
                        nc.tensor.matmul(pe4[:, hh, :], idb_sb[:],
                                         msku_sb[:, hh, :],
                                         start=False, stop=True)
                    deckM = wkE.tile([P, 4, P], BF16, tag="deckM")
                    nc.scalar.activation(deckM[:], pe4[:], AF.Exp)
                    pe4b = pE.tile([P, 4, P], F32, tag="pe4")
                    for hh in range(4):
                        nc.tensor.matmul(pe4b[:, hh, :], ncumT4[:, ts(n, P)],
                                         blkd_sb[:, hh, :],
                                         start=True, stop=False)
                        nc.tensor.matmul(pe4b[:, hh, :], onesA[0:1, :],
                                         cumR[0:1, hh, :],
                                         start=False, stop=False)
                        nc.tensor.matmul(pe4b[:, hh, :], idb_sb[:],
                                         mskud_sb[:, hh, :],
                                         start=False, stop=True)
                    deckP = wkE.tile([P, 4, P], BF16, tag="deckP")
                    nc.scalar.activation(deckP[:], pe4b[:], AF.Exp)
                    # normalized keys kb, their transpose, v transpose
                    prr = pX.tile([P, 2, P], F32, tag="prr")
                    for kh in range(2):
                        nc.tensor.matmul(prr[:, kh, :], onesA[0:1, :],
                                         rkT_sb[0:1, kh, ts(n, P)],
                                         start=True, stop=True)
                    kb = wkE.tile([P, 2, P], BF16, tag="kb")
                    nc.vector.tensor_mul(kb[:], qkv_sb[:, 2:4, ts(n, P)],
                                         prr[:])
                    ptk = pT.tile([P, 4, P], BF16, tag="tr")
                    for kh in range(2):
                        nc.tensor.transpose(ptk[:, kh, :], kb[:, kh, :],
                                            idb_sb[:])
                    kbT = wkE.tile([P, 2, P], BF16, tag="kbT")
                    nc.vector.tensor_copy(kbT[:], ptk[:, 0:2, :])
                    ptv = pT.tile([P, 4, P], BF16, tag="tr")
                    for hh in range(4):
                        nc.tensor.transpose(ptv[:, hh, :],
                                            qkv_sb[:, 4 + hh, ts(n, P)],
                                            idb_sb[:])
                    vT4 = wkE.tile([P, 4, P], BF16, tag="vT4")
                    nc.vector.tensor_copy(vT4[:], ptv[:])
                    # grams and solve matrices (lhsT orientation)
                    pGt = pG.tile([P, 4, P], F32, tag="pG")
                    for kh in range(2):
                        nc.tensor.matmul(pGt[:, kh, :], kb[:, kh, :],
                                         kb[:, kh, :], start=True, stop=True)
                        nc.tensor.matmul(pGt[:, 2 + kh, :], kb[:, kh, :],
                                         qkv_sb[:, kh, ts(n, P)],
                                         start=True, stop=True)
                    grs = wkE.tile([P, 4, P], BF16, tag="grs")
                    nc.vector.tensor_copy(grs[:], pGt[:])
                    MT4 = wkE.tile([P, 4, P], BF16, tag="MT4")
                    PT4 = wkE.tile([P, 4, P], BF16, tag="PT4")
                    for hh2 in range(4):
                        kh = hh2 // 2
                        nc.gpsimd.tensor_mul(MT4[:, hh2, :], grs[:, kh, :],
                                             deckM[:, hh2, :])
                        nc.gpsimd.tensor_mul(PT4[:, hh2, :],
                                             grs[:, 2 + kh, :],
                                             deckP[:, hh2, :])
                    # off-chain per-head prep: beta*v and decay-scaled kbT
                    bv4 = wkE.tile([P, 4, P], BF16, tag="bv4")
                    Ka4 = wkE.tile([P, 4, P], BF16, tag="Ka4")
                    for h in range(4):
                        nc.scalar.activation(bv4[:, h, :], vT4[:, h, :],
                                             AF.Copy,
                                             scale=beta_sb[:, n, h:h + 1])
                        nc.scalar.activation(Ka4[:, h, :], kbT[:, h // 2, :],
                                             AF.Copy,
                                             scale=grd4[:, h:h + 1])
                    # per-head S-chain
                    for h in range(4):
                        kh = h // 2
                        nbeta_col = nbeta_sb[:, n, h:h + 1]
                        qt = qkv_sb[:, kh, ts(n, P)]
                        Sh = S_sb[:, h, :]
                        Shb = S_bf[:, h, :]
                        pc = pC.tile([P, 4, P], F32, tag="pc")
                        nc.tensor.matmul(pc[:, 0, :], qt, Shb, start=True,
                                         stop=True)
                        o_tmp = wkE.tile([P, P], F32, tag="o_tmp")
                        nc.scalar.activation(o_tmp[:], pc[:, 0, :], AF.Copy,
                                             scale=diq4[:, h:h + 1])
                        nc.tensor.matmul(pc[:, 1, :], kb[:, kh, :], Shb,
                                         start=True, stop=True)
                        u0 = uP.tile([P, P], BF16, tag="u")
                        nc.vector.scalar_tensor_tensor(u0[:], pc[:, 1, :],
                                                       dnb4[:, h:h + 1],
                                                       bv4[:, h, :],
                                                       ALU.mult, ALU.add)
                        # Horner: v_{k+1} = u0 - beta*(M v_k)
                        vcur = u0
                        for it in range(3):
                            sl = 2 + (it % 2)
                            nc.tensor.matmul(pc[:, sl, :], MT4[:, h, :],
                                             vcur[:], start=True, stop=True)
                            vnext = uP.tile([P, P], BF16, tag="u")
                            nc.vector.scalar_tensor_tensor(
                                vnext[:], pc[:, sl, :], nbeta_col, u0[:],
                                ALU.mult, ALU.add)
                            vcur = vnext
                        nc.tensor.matmul(pc[:, 3, :], PT4[:, h, :], vcur[:],
                                         start=True, stop=True)
                        o_f = o_sb[:, n, h, :]
                        nc.vector.scalar_tensor_tensor(
                            o_f, pc[:, 3, :], rq_sb[:, n, kh:kh + 1],
                            o_tmp[:], ALU.mult, ALU.add)
                        sqo = wkE.tile([P, P], BF16, tag="sqo")
                        nc.scalar.activation(sqo[:], o_f, AF.Square,
                                             accum_out=ssqo_sb[:, n, h:h + 1])
                        nc.tensor.matmul(pc[:, 0, :], Ka4[:, h, :], vcur[:],
                                         start=True, stop=True)
                        nc.vector.scalar_tensor_tensor(Sh, Sh,
                                                       gend4[:, h:h + 1],
                                                       pc[:, 0, :], ALU.mult,
                                                       ALU.add)
                        nc.scalar.activation(Shb, Sh, AF.Copy)
                # ---- phase F: gated rms norm + out-projection -------------
                for n in range(NCH):
                    l4 = wkEs.tile([P, 4], F32, tag="l4")
                    nc.scalar.activation(l4[:], ssqo_sb[:, n, :], AF.Sqrt,
                                         scale=1.0 / DV, bias=eps_c[:])
                    rro4 = wkEs.tile([P, 4], F32, tag="rro4")
                    nc.vector.reciprocal(rro4[:], l4[:])
                    og3 = wkE.tile([P, 512], BF16, tag="og3")
                    for h in range(4):
                        nc.vector.tensor_scalar(og3[:, ts(h, P)],
                                                o_sb[:, n, h, :],
                                                rro4[:, h:h + 1], None,
                                                ALU.mult)
                    og4 = wkE.tile([P, 512], BF16, tag="og4")
                    nc.gpsimd.tensor_mul(og4[:], og3[:], zs_sb[:, n, :])
                    ptF = pT.tile([P, 4, P], BF16, tag="tr")
                    for h in range(4):
                        nc.tensor.transpose(ptF[:, h, :], og4[:, ts(h, P)],
                                            idb_sb[:])
                    nc.vector.tensor_copy(ogT_sb[:, :, ts(n, P)], ptF[:])
                    ob = wkE.tile([P, D], BF16, tag="ob")
                    for nn in range(4):
                        po = pF.tile([P, 512], F32)
                        for jp in range(2):
                            nc.tensor.matmul(
                                po[:], ogT_sb[:, 2 * jp:2 * jp + 2, ts(n, P)],
                                wout_sb[:, 2 * jp:2 * jp + 2, ts(nn, 512)],
                                start=(jp == 0), stop=(jp == 1), perf_mode=DR)
                        if nn == 1:
                            nc.scalar.activation(ob[:, ts(nn, 512)], po[:],
                                                 AF.Copy)
                        else:
                            nc.vector.tensor_copy(ob[:, ts(nn, 512)], po[:])
                    nc.sync.dma_start(p2[ts(n, P), :], ob[:])
    nc.compile()
    return nc


# ============================================================ host helpers
def _bf(a):
    return np.ascontiguousarray(a.astype(BFNP))


def _f8(a):
    return np.ascontiguousarray(a.astype(F8NP))


def _perm_kt(w):
    """[KT*P, C] row-tiled -> [P, KT*C] partition-major."""
    kt = w.shape[0] // P
    c = w.shape[1]
    return np.ascontiguousarray(
        w.reshape(kt, P, c).transpose(1, 0, 2).reshape(P, kt * c))


def _rms_scale(x2):
    ssq = np.sum(x2.astype(np.float64) ** 2, axis=1)
    return (1.0 / np.sqrt(ssq / D + EPS)).astype(np.float32)


def _prep_attn_inputs(x, input_pos, ln1_w, q_w, k_w, v_w, o_w, qn_w, kn_w):
    x2 = x.reshape(T, D).astype(np.float32)
    ln1f = (1.0 + ln1_w.astype(np.float32))
    s1 = _rms_scale(x2)
    xh = x2 * s1[:, None]
    xT_np = _f8(_perm_kt(xh.T)).reshape(P, KT, T)
    inv_freq = 1.0 / THETA ** (np.arange(0, ROT, 2, dtype=np.float32) / ROT)
    fr = input_pos.astype(np.float32)[:, None] * inv_freq[None, :]
    cos = np.cos(fr).astype(np.float32); sin = np.sin(fr).astype(np.float32)
    cs = np.concatenate([np.tile(cos, (1, 3)), np.tile(sin, (1, 3))], axis=1)
    csd_np = _perm_kt(cs).reshape(P, TT, 96).astype(np.float32)
    qk1_np = _bf(np.concatenate(
        [np.tile(1.0 + qn_w[None, :], (P, 1)),
         np.tile(1.0 + kn_w[None, :], (P, 1))], axis=1))
    a = np.arange(P)[:, None]; b = np.arange(512)[None, :]
    m4_np = _bf(np.concatenate(
        [(a + 128 * r <= b).astype(np.float32) for r in range(4)], axis=1))
    idm_np = _bf(np.eye(P, dtype=np.float32))
    ins = []
    for c in range(NCORE):
        qh = [2 * c, 2 * c + 1]; kvh = c // 2
        qrows = np.concatenate([q_w[h * 256: h * 256 + 128] for h in qh]
                               + [q_w[h * 256 + 128: h * 256 + 256] for h in qh])
        wqg_np = _f8(_perm_kt((qrows * ln1f[None, :]).T) * SW
                     ).reshape(P, KT, 512)
        kvrows = np.concatenate([k_w[kvh * 128: kvh * 128 + 128],
                                 v_w[kvh * 128: kvh * 128 + 128]])
        wkv_np = _f8(_perm_kt((kvrows * ln1f[None, :]).T) * SW
                     ).reshape(P, KT, 256)
        wo_np = _f8(_perm_kt(o_w[:, 2 * c * 128: 2 * c * 128 + 256].T) * SW
                    ).reshape(P, 2, D)
        ins.append(dict(xT=xT_np, wqg=wqg_np, wkv=wkv_np, wo=wo_np,
                        csd=csd_np, qk1=qk1_np, m4=m4_np, idm=idm_np))
    return ins


def _prep_delta_inputs(h, ln2_w, dn_qkv_w, dn_z_w, dn_b_w, dn_a_w, conv_w,
                       dt_bias, A_log, dn_norm_w, dn_out_w):
    ln2f = (1.0 + ln2_w.astype(np.float32))
    h2 = h.astype(np.float32)
    s2 = _rms_scale(h2)
    hh = h2 * s2[:, None]
    hT_np = _f8(_perm_kt(hh.T)).reshape(P, KT, T)
    a2 = np.arange(P)
    # -inf-style bias tiles: -60000 where the causal mask zeroes the entry
    # (added to the log-decay tile before exp -> exact 0), 0 where valid.
    msku_np = _bf(np.tile(
        np.where(a2[:, None] >= a2[None, :], -60000.0, 0.0
                 ).astype(np.float32), (1, 4))).reshape(P, 4, P)
    mskud_np = _bf(np.tile(
        np.where(a2[:, None] > a2[None, :], -60000.0, 0.0
                 ).astype(np.float32), (1, 4))).reshape(P, 4, P)
    idb_np = _bf(np.eye(P, dtype=np.float32))
    idf_np = np.eye(P, dtype=np.float32)
    nw_np = _bf(np.tile(dn_norm_w.astype(np.float32)[None, :], (P, 4)))
    blk = np.zeros((4, 4, P), dtype=np.float32)
    for hh2 in range(4):
        blk[hh2, hh2, :] = 1.0
    blkd_np = np.ascontiguousarray(blk.reshape(4, 512))
    ins = []
    for c in range(NCORE):
        khs = [2 * c, 2 * c + 1]
        vhs = [4 * c + j for j in range(4)]
        qrows = np.concatenate([dn_qkv_w[kh * DK:(kh + 1) * DK] for kh in khs])
        krows = np.concatenate([dn_qkv_w[KEY_DIM + kh * DK:
                                         KEY_DIM + (kh + 1) * DK] for kh in khs])
        vrows = dn_qkv_w[2 * KEY_DIM + vhs[0] * DV:
                         2 * KEY_DIM + (vhs[-1] + 1) * DV]
        rows = np.concatenate([qrows, krows, vrows])  # [1024, D]
        wqkv_np = _f8(_perm_kt((rows * ln2f[None, :]).T) * SW
                      ).reshape(P, KT, 1024)
        crow_q = np.concatenate([conv_w[kh * DK:(kh + 1) * DK, 0, :]
                                 for kh in khs])
        crow_k = np.concatenate([conv_w[KEY_DIM + kh * DK:
                                        KEY_DIM + (kh + 1) * DK, 0, :]
                                 for kh in khs])
        crow_v = conv_w[2 * KEY_DIM + vhs[0] * DV:
                        2 * KEY_DIM + (vhs[-1] + 1) * DV, 0, :]
        crows = np.concatenate([crow_q, crow_k, crow_v])  # [1024, 4]
        cwt_np = np.ascontiguousarray(
            crows.reshape(8, P, KCONV).transpose(1, 0, 2).reshape(P, 8 * KCONV)
        ).astype(np.float32) * np.float32(ISW)
        zrows = dn_z_w[vhs[0] * DV:(vhs[-1] + 1) * DV]
        wz_np = _f8(_perm_kt((zrows * ln2f[None, :]).T) * SW
                    ).reshape(P, KT, 512)
        abrows = np.concatenate([dn_a_w[vhs[0]:vhs[-1] + 1],
                                 dn_b_w[vhs[0]:vhs[-1] + 1]])
        wab_np = _f8(_perm_kt((abrows * ln2f[None, :]).T) * SW
                     ).reshape(P, KT, 8)
        wout_np = _f8(_perm_kt(dn_out_w[:, vhs[0] * DV:(vhs[-1] + 1) * DV].T)
                      * SW).reshape(P, 4, D)
        dtb_np = np.tile(dt_bias[vhs[0]:vhs[-1] + 1][None, :],
                         (P, 1)).astype(np.float32)
        nega_np = np.tile(-np.exp(A_log[vhs[0]:vhs[-1] + 1])[None, :],
                          (P, TT)).astype(np.float32).reshape(P, TT, 4)
        ins.append(dict(hT=hT_np, wqkv=wqkv_np, cwt=cwt_np, wz=wz_np,
                        wab=wab_np, wout=wout_np,
                        dtb=dtb_np, nega=nega_np, nwbc=nw_np,
                        msku=msku_np, mskud=mskud_np, blkd=blkd_np,
                        idb=idb_np, idf=idf_np))
    return ins


_CACHE = {}


def _get_attn_nc():
    if "attn" not in _CACHE:
        _CACHE["attn"] = build_attn()
    return _CACHE["attn"]


def _get_delta_nc():
    if "delta" not in _CACHE:
        _CACHE["delta"] = build_delta()
    return _CACHE["delta"]


def run_delta(h, ln2_w, dn_qkv_w, dn_z_w, dn_b_w, dn_a_w, conv_w,
              dt_bias, A_log, dn_norm_w, dn_out_w):
    nc2 = _get_delta_nc()
    ins2 = _prep_delta_inputs(h, ln2_w, dn_qkv_w, dn_z_w, dn_b_w, dn_a_w,
                              conv_w, dt_bias, A_log, dn_norm_w, dn_out_w)
    res2 = run_bass_kernel_spmd(nc2, ins2, core_ids=list(range(NCORE)))
    out = h.astype(np.float32).copy()
    for c in range(NCORE):
        out += res2.results[c]["p2"].astype(np.float32) * np.float32(ISW)
    return out


def kernel(x, input_pos, ln1_w, ln2_w, q_w, k_w, v_w, o_w, qn_w, kn_w,
           dn_qkv_w, dn_z_w, dn_b_w, dn_a_w, conv_w, dt_bias, A_log,
           dn_norm_w, dn_out_w):
    x = np.asarray(x); input_pos = np.asarray(input_pos)
    nc1 = _get_attn_nc()
    ins1 = _prep_attn_inputs(x, input_pos, np.asarray(ln1_w),
                             np.asarray(q_w), np.asarray(k_w),
                             np.asarray(v_w), np.asarray(o_w),
                             np.asarray(qn_w), np.asarray(kn_w))
    res1 = run_bass_kernel_spmd(nc1, ins1, core_ids=list(range(NCORE)))
    h = x.reshape(T, D).astype(np.float32).copy()
    for c in range(NCORE):
        h += res1.results[c]["p1"].astype(np.float32) * np.float32(ISW)

    out = run_delta(h, np.asarray(ln2_w), np.asarray(dn_qkv_w),
                    np.asarray(dn_z_w), np.asarray(dn_b_w),
                    np.asarray(dn_a_w), np.asarray(conv_w),
                    np.asarray(dt_bias), np.asarray(A_log),
                    np.asarray(dn_norm_w), np.asarray(dn_out_w))
    return out.reshape(B, T, D).astype(np.float32)
